# revision 30
# baseline (speedup 1.0000x reference)
"""Trainium2 Bass kernel: out = x @ ((W_int + offset) * scale), fp8 DoubleRow.

Math (same quantization as the 343us N-sharded baseline): V = W - 63
(|V| <= 63), cast to fp8 e4m3; x cast to fp8 e4m3. Then
  out[m,n] = scale[n] * ((x8 @ V8)[m,n] + (63 + offset[n]) * rowsum(x)[m])
with the rank-1 term in exact f32 (DVE STT), so only x/V carry fp8 error.
Measured rel err 1.8524e-2 vs f64 (threshold 2e-2; the extra 7.5e-5 over
the baseline's 1.8449e-2 is the bf16 output staging).

Changes vs the N-sharded baseline (measured 338.4us):
- Shard along M instead of N: each core owns m-cols [c*512, (c+1)*512) and
  ALL of N. N = 11008 = 86*128 exactly, so there are no padded stationary
  blocks: 86 nb x 16 kp = 1376 DoubleRow MMs per core (vs 1408 padded), a
  2.3% shorter PE stream (297us floor at the measured 216ns/512-col MM).
  W (45MB fp8) streams per-nb (512KB tiles, 148 GB/s) instead of resident.
- Warmup: ~50 dummy DR MMs on memset tiles keep the PE busy from ~7.5us so
  the HAM clock-gate reaches K=8/8 by ~11us (baseline ran at 1.2GHz until
  30us, ~10us penalty). First W/x DMAs are split into 64-256KB pieces over
  all three DMA queues (sync/scalar/gpsimd) in consumption order — early
  per-queue DMA only sustains ~60-100GB/s, so the ~2.5MB critical mass
  (w0+w1+x) gates the real stream start at ~13-15us regardless.
- Output staged bf16 (halves out DMA, +0.004% err), out DMAs ride the
  scalar HWDGE queue right after each ACT (the gpsimd SWDGE queue fell
  ~8.6us behind in an earlier revision). Last group runs as two m-halves
  in separate PSUM banks so its epilogue overlaps the final MMs.

Per-core schedule: for nb in 0..85: 16 kp MMs accumulate into one PSUM
bank ([128n, 512m], stationary w[nb][:,kp] [128,2,128], moving x
[128,2,512]); epilogue: DVE STT adds rowsum*(63+off), ACT applies scale ->
bf16 SBUF, DMA to DRAM outt[(nb p), m]. Host transposes + concatenates the
8 m-slices. Measured 320.7-323us HW exec over 5 clean runs (vs 338.4
baseline, -5%); occasional runs show a chip-level P0 downclock to 2.0GHz
(PE dur 454ns vs 379) that adds ~60us regardless of kernel structure.
"""

import numpy as np
import ml_dtypes

M, K, N = 4096, 4096, 11008
NCORES = 8
MSH = M // NCORES          # 512 m-cols per core
P = 128
KP = 16                    # k-pairs of 256
NB = N // P                # 86 n-blocks, exact
WBUFS = 8                  # streamed W tiles in flight
NWARM = 50                 # dummy HAM-warmup matmuls (bridge until real data)

_E4 = ml_dtypes.float8_e4m3

_cache = {}


def _build_nc():
    import concourse.bacc as bacc
    import concourse.mybir as mybir
    import concourse.tile as tile

    fp8 = mybir.dt.float8e4
    f32 = mybir.dt.float32
    bf16 = mybir.dt.bfloat16
    DR = mybir.MatmulPerfMode.DoubleRow
    Copy = mybir.ActivationFunctionType.Copy

    nc = bacc.Bacc(None, target_bir_lowering=False)
    # xq partition-major: [p, kp*2*MSH] so each DMA piece reads >=2KB
    # contiguous per partition (x^T fp8 pairs, this core's m)
    xq = nc.dram_tensor("xq", [P, KP * 2 * MSH], fp8, kind="ExternalInput")
    # wq rows: nb*P + p ; cols: kp*256 + slot*128 + nn  (full W, fp8 pairs)
    wq = nc.dram_tensor("wq", [NB * P, KP * 2 * P], fp8, kind="ExternalInput")
    sbc = nc.dram_tensor("sbc", [P, MSH], f32, kind="ExternalInput")
    offc = nc.dram_tensor("offc", [P, NB], f32, kind="ExternalInput")
    scalec = nc.dram_tensor("scalec", [P, NB], f32, kind="ExternalInput")
    outt = nc.dram_tensor("outt", [NB * P, MSH], bf16, kind="ExternalOutput")

    xq4 = xq.ap().rearrange(
        "p (kp s m) -> p kp s m", kp=KP, s=2
    )                                                      # [128,16,2,512]
    wq3 = wq.ap().rearrange("(nb p) f -> p nb f", p=P)     # [128, 86, 4096]
    outt3 = outt.ap().rearrange("(nb p) m -> p nb m", p=P)  # [128, 86, 512]

    with tile.TileContext(nc) as tc:
        with (
            tc.tile_pool(name="wpool", bufs=WBUFS) as wpool,
            tc.tile_pool(name="xpool", bufs=1) as xpool,
            tc.tile_pool(name="cpool", bufs=1) as cpool,
            tc.tile_pool(name="opool", bufs=3) as opool,
            tc.tile_pool(name="psp", bufs=4, space="PSUM") as psp,
            tc.tile_pool(name="pswarm", bufs=1, space="PSUM") as pswarm,
        ):
            # --- HAM warmup: memset junk tiles, dummy MMs keep PE busy ---
            wm = cpool.tile([P, 2, P], fp8, tag="wm")
            xm = cpool.tile([P, 2, P], fp8, tag="xm")
            nc.gpsimd.memset(wm[:], 0)
            nc.gpsimd.memset(xm[:], 0)
            psw = pswarm.tile([P, P], f32, tag="psw")

            def dummy_mms(n):
                for _ in range(n):
                    nc.tensor.matmul(
                        psw[:], wm[:], xm[:],
                        start=True, stop=True, perf_mode=DR,
                    )

            dummy_mms(NWARM)

            # --- first-wave DMAs: small pieces, issue order = need order ---
            x_sb = []
            for c in range(4):
                x_sb.append(
                    xpool.tile([P, 4, 2, MSH], fp8, tag=f"xc{c}", name=f"x{c}")
                )

            def load_x(c, k0, nk, eng):
                eng.dma_start(
                    x_sb[c][:, k0:k0 + nk, :, :],
                    xq4[:, 4 * c + k0:4 * c + k0 + nk, :, :],
                )

            w_sb = [
                wpool.tile([P, KP, 2, P], fp8, tag="w", name=f"w{i}")
                for i in range(2)
            ]

            def load_wq(nb, q, eng):  # one kp-quarter of w[nb]
                eng.dma_start(
                    w_sb[nb][:, 4 * q:4 * q + 4, :, :],
                    wq3[:, nb, 1024 * q:1024 * (q + 1)].rearrange(
                        "p (k s n) -> p k s n", k=4, s=2
                    ),
                )

            # sync: interleave w0 quarters with x pieces, arrival tracks the
            # cold-MM consumption order (w0.q_i covers kp4i..4i+3).
            load_wq(0, 0, nc.sync)
            load_x(0, 0, 1, nc.sync)    # kp0
            load_x(0, 1, 1, nc.sync)    # kp1
            load_wq(0, 1, nc.sync)
            load_x(0, 2, 2, nc.sync)    # kp2-3
            load_x(1, 0, 2, nc.sync)    # kp4-5
            load_wq(0, 2, nc.sync)
            load_x(1, 2, 2, nc.sync)    # kp6-7
            load_wq(0, 3, nc.sync)
            load_x(2, 0, 2, nc.sync)    # kp8-9
            load_x(2, 2, 2, nc.sync)    # kp10-11

            # gpsimd (free after memsets): the last-consumed x pieces
            load_x(3, 0, 2, nc.gpsimd)  # kp12-13
            load_x(3, 2, 2, nc.gpsimd)  # kp14-15

            # scalar (after its ACT table load): w1 halves + constants
            for h in range(2):
                nc.scalar.dma_start(
                    w_sb[1][:, 8 * h:8 * h + 8, :, :],
                    wq3[:, 1, 2048 * h:2048 * (h + 1)].rearrange(
                        "p (k s n) -> p k s n", k=8, s=2
                    ),
                )
            sbc_sb = cpool.tile([P, MSH], f32, tag="sbc")
            nc.scalar.dma_start(sbc_sb[:], sbc.ap())
            offc_sb = cpool.tile([P, NB], f32, tag="offc")
            nc.scalar.dma_start(offc_sb[:], offc.ap())
            scalec_sb = cpool.tile([P, NB], f32, tag="scalec")
            nc.scalar.dma_start(scalec_sb[:], scalec.ap())

            # remaining W stream on sync, paced by wpool buf releases
            def load_w(nb):
                t = wpool.tile([P, KP, 2, P], fp8, tag="w", name=f"w{nb}")
                nc.sync.dma_start(
                    t[:],
                    wq3[:, nb, :].rearrange("p (k s n) -> p k s n", k=KP, s=2),
                )
                w_sb.append(t)

            for nb in range(2, WBUFS):
                load_w(nb)

            # --- main loop: 86 nb groups; the last one in two m-halves so
            # its epilogue overlaps the final MMs ---
            def epilogue(nb, ps, m0, m1):
                # ps += (63 + offset[n]) * rowsum_x[m]
                nc.vector.scalar_tensor_tensor(
                    ps[:],
                    sbc_sb[:, m0:m1],
                    offc_sb[:, nb:nb + 1],
                    ps[:],
                    mybir.AluOpType.mult,
                    mybir.AluOpType.add,
                )
                o_sb = opool.tile([P, m1 - m0], bf16, tag="o")
                nc.scalar.activation(
                    o_sb[:], ps[:], Copy,
                    scale=scalec_sb[:, nb:nb + 1],
                )
                nc.scalar.dma_start(outt3[:, nb, m0:m1], o_sb[:])

            def mm(ps, nb, kp, m0=0, m1=MSH):
                nc.tensor.matmul(
                    ps[:],
                    w_sb[nb][:, kp, :, :],
                    x_sb[kp // 4][:, kp % 4, :, m0:m1],
                    start=(kp == 0),
                    stop=(kp == KP - 1),
                    perf_mode=DR,
                )

            # main loop; last group in two m-halves in SEPARATE PSUM banks
            # so its epilogue overlaps the final MMs
            for nb in range(NB):
                if WBUFS + nb < NB:
                    load_w(WBUFS + nb)
                last = nb == NB - 1
                halves = ((0, MSH),) if not last else ((0, 256), (256, MSH))
                for m0, m1 in halves:
                    ps = psp.tile([P, m1 - m0], f32, tag="ps")
                    for kp in range(KP):
                        mm(ps, nb, kp, m0, m1)
                    epilogue(nb, ps, m0, m1)
    nc.compile()
    return nc


def _get_nc():
    if "nc" not in _cache:
        _cache["nc"] = _build_nc()
    return _cache["nc"]


def _prep_inputs(x, weight, antiquant_scale, antiquant_offset):
    x = np.asarray(x, dtype=np.float32)
    weight = np.asarray(weight)
    scale = np.asarray(antiquant_scale, dtype=np.float32)
    off = np.asarray(antiquant_offset, dtype=np.float32)

    xt8 = np.ascontiguousarray(x.astype(_E4).T)         # [K, M] fp8
    rs = x.astype(np.float64).sum(axis=1).astype(np.float32)

    V8 = (weight.astype(np.float32) - 63.0).astype(_E4)  # [K, N]
    # rows (nb, p), cols (kp, slot, nn); k = kp*256 + slot*128 + p
    wdr = np.ascontiguousarray(
        V8.reshape(KP, 2, P, NB, P).transpose(3, 2, 0, 1, 4)
    ).reshape(NB * P, KP * 2 * P)
    offc = np.ascontiguousarray((63.0 + off).reshape(NB, P).T)
    scalec = np.ascontiguousarray(scale.reshape(NB, P).T)

    in_maps = []
    for c in range(NCORES):
        sl = slice(c * MSH, (c + 1) * MSH)
        xdr = np.ascontiguousarray(
            xt8[:, sl].reshape(KP, 2, P, MSH).transpose(2, 0, 1, 3)
        ).reshape(P, KP * 2 * MSH)
        sbc = np.ascontiguousarray(
            np.broadcast_to(rs[sl][None, :], (P, MSH))
        )
        in_maps.append({
            "xq": xdr,
            "wq": wdr,
            "sbc": sbc,
            "offc": offc,
            "scalec": scalec,
        })
    return in_maps


def kernel(x, weight, antiquant_scale, antiquant_offset, _trace=False):
    from concourse.bass_utils import run_bass_kernel_spmd

    nc = _get_nc()
    in_maps = _prep_inputs(x, weight, antiquant_scale, antiquant_offset)
    res = run_bass_kernel_spmd(
        nc, in_maps, core_ids=list(range(NCORES)), trace=_trace
    )
    out = np.empty((M, N), dtype=np.float32)
    for c in range(NCORES):
        outt = np.asarray(res.results[c]["outt"])      # [N, MSH] bf16
        out[c * MSH:(c + 1) * MSH, :] = outt.T.astype(np.float32)
    if _trace:
        _cache["last_result"] = res
    return out


# revision 33
# speedup vs baseline: 1.0072x; 1.0072x over previous
"""Trainium2 Bass kernel: out = x @ ((W_int + offset) * scale), fp8 DoubleRow.

Math (same quantization as the 343us N-sharded baseline): V = W - 63
(|V| <= 63), cast to fp8 e4m3; x cast to fp8 e4m3. Then
  out[m,n] = scale[n] * ((x8 @ V8)[m,n] + (63 + offset[n]) * rowsum(x)[m])
with the rank-1 term in exact f32 (DVE STT), so only x/V carry fp8 error.
Measured rel err 1.8524e-2 vs f64 (threshold 2e-2; the extra 7.5e-5 over
the baseline's 1.8449e-2 is the bf16 output staging).

Changes vs the N-sharded baseline (measured 338.4us):
- Shard along M instead of N: each core owns m-cols [c*512, (c+1)*512) and
  ALL of N. N = 11008 = 86*128 exactly, so there are no padded stationary
  blocks: 86 nb x 16 kp = 1376 DoubleRow MMs per core (vs 1408 padded), a
  2.3% shorter PE stream (297us floor at the measured 216ns/512-col MM).
  W (45MB fp8) streams per-nb (512KB tiles, 148 GB/s) instead of resident.
- Warmup: ~50 dummy DR MMs on memset tiles keep the PE busy from ~7.5us so
  the HAM clock-gate reaches K=8/8 by ~11us (baseline ran at 1.2GHz until
  30us, ~10us penalty). First W/x DMAs are split into 64-256KB pieces over
  all three DMA queues (sync/scalar/gpsimd) in consumption order — early
  per-queue DMA only sustains ~60-100GB/s, so the ~2.5MB critical mass
  (w0+w1+x) gates the real stream start at ~13-15us regardless.
- Output staged bf16 (halves out DMA, +0.004% err), out DMAs ride the
  scalar HWDGE queue right after each ACT (the gpsimd SWDGE queue fell
  ~8.6us behind in an earlier revision). Last group runs as two m-halves
  in separate PSUM banks so its epilogue overlaps the final MMs.

Per-core schedule: for nb in 0..85: 16 kp MMs accumulate into one PSUM
bank ([128n, 512m], stationary w[nb][:,kp] [128,2,128], moving x
[128,2,512]); epilogue: DVE STT adds rowsum*(63+off), ACT applies scale ->
bf16 SBUF, DMA to DRAM outt[(nb p), m]. Host transposes + concatenates the
8 m-slices. Measured 320.7-323us HW exec over 5 clean runs (vs 338.4
baseline, -5%); occasional runs show a chip-level P0 downclock to 2.0GHz
(PE dur 454ns vs 379) that adds ~60us regardless of kernel structure.
"""

import numpy as np
import ml_dtypes

M, K, N = 4096, 4096, 11008
NCORES = 8
MSH = M // NCORES          # 512 m-cols per core
P = 128
KP = 16                    # k-pairs of 256
NB = N // P                # 86 n-blocks, exact
WBUFS = 8                  # streamed W tiles in flight
NWARM = 56                 # dummy HAM-warmup matmuls (bridge until real data)

_E4 = ml_dtypes.float8_e4m3

_cache = {}


def _build_nc():
    import concourse.bacc as bacc
    import concourse.mybir as mybir
    import concourse.tile as tile

    fp8 = mybir.dt.float8e4
    f32 = mybir.dt.float32
    bf16 = mybir.dt.bfloat16
    DR = mybir.MatmulPerfMode.DoubleRow
    Copy = mybir.ActivationFunctionType.Copy

    nc = bacc.Bacc(None, target_bir_lowering=False)
    # xq partition-major: [p, kp*2*MSH] so each DMA piece reads >=2KB
    # contiguous per partition (x^T fp8 pairs, this core's m)
    xq = nc.dram_tensor("xq", [P, KP * 2 * MSH], fp8, kind="ExternalInput")
    # wq rows: nb*P + p ; cols: kp*256 + slot*128 + nn  (full W, fp8 pairs)
    wq = nc.dram_tensor("wq", [NB * P, KP * 2 * P], fp8, kind="ExternalInput")
    sbc = nc.dram_tensor("sbc", [P, MSH], f32, kind="ExternalInput")
    offc = nc.dram_tensor("offc", [P, NB], f32, kind="ExternalInput")
    scalec = nc.dram_tensor("scalec", [P, NB], f32, kind="ExternalInput")
    outt = nc.dram_tensor("outt", [NB * P, MSH], bf16, kind="ExternalOutput")

    xq4 = xq.ap().rearrange(
        "p (kp s m) -> p kp s m", kp=KP, s=2
    )                                                      # [128,16,2,512]
    wq3 = wq.ap().rearrange("(nb p) f -> p nb f", p=P)     # [128, 86, 4096]
    outt3 = outt.ap().rearrange("(nb p) m -> p nb m", p=P)  # [128, 86, 512]

    with tile.TileContext(nc) as tc:
        with (
            tc.tile_pool(name="wpool", bufs=WBUFS) as wpool,
            tc.tile_pool(name="xpool", bufs=1) as xpool,
            tc.tile_pool(name="cpool", bufs=1) as cpool,
            tc.tile_pool(name="opool", bufs=3) as opool,
            tc.tile_pool(name="psp", bufs=4, space="PSUM") as psp,
            tc.tile_pool(name="pswarm", bufs=1, space="PSUM") as pswarm,
        ):
            # --- HAM warmup: dummy MMs on junk tiles keep the PE busy from
            # ~6.5us (right after its preamble). Memsets ride the otherwise
            # idle vector engine so gpsimd can issue its x pieces at once. ---
            wm = cpool.tile([P, 2, P], fp8, tag="wm")
            xm = cpool.tile([P, 2, P], fp8, tag="xm")
            nc.vector.memset(wm[:], 0)
            nc.vector.memset(xm[:], 0)
            psw = pswarm.tile([P, P], f32, tag="psw")

            def dummy_mms(n):
                for _ in range(n):
                    nc.tensor.matmul(
                        psw[:], wm[:], xm[:],
                        start=True, stop=True, perf_mode=DR,
                    )

            dummy_mms(NWARM)

            # --- first-wave DMAs: small pieces, issue order = need order ---
            x_sb = []
            for c in range(4):
                x_sb.append(
                    xpool.tile([P, 4, 2, MSH], fp8, tag=f"xc{c}", name=f"x{c}")
                )

            def load_x(c, k0, nk, eng):
                eng.dma_start(
                    x_sb[c][:, k0:k0 + nk, :, :],
                    xq4[:, 4 * c + k0:4 * c + k0 + nk, :, :],
                )

            w_sb = [
                wpool.tile([P, KP, 2, P], fp8, tag="w", name=f"w{i}")
                for i in range(2)
            ]

            def load_wq(nb, q, eng):  # one kp-quarter of w[nb]
                eng.dma_start(
                    w_sb[nb][:, 4 * q:4 * q + 4, :, :],
                    wq3[:, nb, 1024 * q:1024 * (q + 1)].rearrange(
                        "p (k s n) -> p k s n", k=4, s=2
                    ),
                )

            # sync: interleave w0 quarters with x pieces, arrival tracks the
            # cold-MM consumption order (w0.q_i covers kp4i..4i+3).
            load_wq(0, 0, nc.sync)
            load_x(0, 0, 1, nc.sync)    # kp0
            load_x(0, 1, 1, nc.sync)    # kp1
            load_wq(0, 1, nc.sync)
            load_x(0, 2, 2, nc.sync)    # kp2-3
            load_x(1, 0, 2, nc.sync)    # kp4-5
            load_wq(0, 2, nc.sync)
            load_x(1, 2, 2, nc.sync)    # kp6-7
            load_wq(0, 3, nc.sync)
            load_x(2, 0, 2, nc.sync)    # kp8-9
            load_x(2, 2, 2, nc.sync)    # kp10-11

            # gpsimd (free after memsets): the last-consumed x pieces
            load_x(3, 0, 2, nc.gpsimd)  # kp12-13
            load_x(3, 2, 2, nc.gpsimd)  # kp14-15

            # scalar (after its ACT table load): w1 halves + constants
            for h in range(2):
                nc.scalar.dma_start(
                    w_sb[1][:, 8 * h:8 * h + 8, :, :],
                    wq3[:, 1, 2048 * h:2048 * (h + 1)].rearrange(
                        "p (k s n) -> p k s n", k=8, s=2
                    ),
                )
            sbc_sb = cpool.tile([P, MSH], f32, tag="sbc")
            nc.scalar.dma_start(sbc_sb[:], sbc.ap())
            offc_sb = cpool.tile([P, NB], f32, tag="offc")
            nc.scalar.dma_start(offc_sb[:], offc.ap())
            scalec_sb = cpool.tile([P, NB], f32, tag="scalec")
            nc.scalar.dma_start(scalec_sb[:], scalec.ap())

            # remaining W stream on sync, paced by wpool buf releases
            def load_w(nb):
                t = wpool.tile([P, KP, 2, P], fp8, tag="w", name=f"w{nb}")
                nc.sync.dma_start(
                    t[:],
                    wq3[:, nb, :].rearrange("p (k s n) -> p k s n", k=KP, s=2),
                )
                w_sb.append(t)

            for nb in range(2, WBUFS):
                load_w(nb)

            # --- main loop: 86 nb groups; the last one in two m-halves so
            # its epilogue overlaps the final MMs ---
            def epilogue(nb, ps, m0, m1):
                # ps += (63 + offset[n]) * rowsum_x[m]
                nc.vector.scalar_tensor_tensor(
                    ps[:],
                    sbc_sb[:, m0:m1],
                    offc_sb[:, nb:nb + 1],
                    ps[:],
                    mybir.AluOpType.mult,
                    mybir.AluOpType.add,
                )
                o_sb = opool.tile([P, m1 - m0], bf16, tag="o")
                nc.scalar.activation(
                    o_sb[:], ps[:], Copy,
                    scale=scalec_sb[:, nb:nb + 1],
                )
                nc.scalar.dma_start(outt3[:, nb, m0:m1], o_sb[:])

            def mm(ps, nb, kp, m0=0, m1=MSH):
                nc.tensor.matmul(
                    ps[:],
                    w_sb[nb][:, kp, :, :],
                    x_sb[kp // 4][:, kp % 4, :, m0:m1],
                    start=(kp == 0),
                    stop=(kp == KP - 1),
                    perf_mode=DR,
                )

            # main loop; last group in two m-halves in SEPARATE PSUM banks
            # so its epilogue overlaps the final MMs
            for nb in range(NB):
                if WBUFS + nb < NB:
                    load_w(WBUFS + nb)
                last = nb == NB - 1
                halves = ((0, MSH),) if not last else ((0, 256), (256, MSH))
                for m0, m1 in halves:
                    ps = psp.tile([P, m1 - m0], f32, tag="ps")
                    for kp in range(KP):
                        mm(ps, nb, kp, m0, m1)
                    epilogue(nb, ps, m0, m1)
    nc.compile()
    return nc


def _get_nc():
    if "nc" not in _cache:
        _cache["nc"] = _build_nc()
    return _cache["nc"]


def _prep_inputs(x, weight, antiquant_scale, antiquant_offset):
    x = np.asarray(x, dtype=np.float32)
    weight = np.asarray(weight)
    scale = np.asarray(antiquant_scale, dtype=np.float32)
    off = np.asarray(antiquant_offset, dtype=np.float32)

    xt8 = np.ascontiguousarray(x.astype(_E4).T)         # [K, M] fp8
    rs = x.astype(np.float64).sum(axis=1).astype(np.float32)

    V8 = (weight.astype(np.float32) - 63.0).astype(_E4)  # [K, N]
    # rows (nb, p), cols (kp, slot, nn); k = kp*256 + slot*128 + p
    wdr = np.ascontiguousarray(
        V8.reshape(KP, 2, P, NB, P).transpose(3, 2, 0, 1, 4)
    ).reshape(NB * P, KP * 2 * P)
    offc = np.ascontiguousarray((63.0 + off).reshape(NB, P).T)
    scalec = np.ascontiguousarray(scale.reshape(NB, P).T)

    in_maps = []
    for c in range(NCORES):
        sl = slice(c * MSH, (c + 1) * MSH)
        xdr = np.ascontiguousarray(
            xt8[:, sl].reshape(KP, 2, P, MSH).transpose(2, 0, 1, 3)
        ).reshape(P, KP * 2 * MSH)
        sbc = np.ascontiguousarray(
            np.broadcast_to(rs[sl][None, :], (P, MSH))
        )
        in_maps.append({
            "xq": xdr,
            "wq": wdr,
            "sbc": sbc,
            "offc": offc,
            "scalec": scalec,
        })
    return in_maps


def kernel(x, weight, antiquant_scale, antiquant_offset, _trace=False):
    from concourse.bass_utils import run_bass_kernel_spmd

    nc = _get_nc()
    in_maps = _prep_inputs(x, weight, antiquant_scale, antiquant_offset)
    res = run_bass_kernel_spmd(
        nc, in_maps, core_ids=list(range(NCORES)), trace=_trace
    )
    out = np.empty((M, N), dtype=np.float32)
    for c in range(NCORES):
        outt = np.asarray(res.results[c]["outt"])      # [N, MSH] bf16
        out[c * MSH:(c + 1) * MSH, :] = outt.T.astype(np.float32)
    if _trace:
        _cache["last_result"] = res
    return out
